# revision 25
# baseline (speedup 1.0000x reference)
"""DiceEmbedding kernel for 8 Trainium2 NeuronCores.

Reference math (per element v of batch_val [262144]):
    theta    = ln(0.01 + |v|) / 85 * pi
    s, c     = sin(theta), cos(theta)
    polar    = [c, s*c, s^2*c, ..., s^8*c, s^10]           # [10]
    out      = (polar @ Q.T) @ W.T + b                     # [1024]

Host folds Q/W/b into one weight:  Wq = W @ Q  [1024, 10], and appends an
ones-row so the bias rides along row 10 of an [11, 1024] rhs.

Per-core device program (data-parallel over N: 32768 elems per core):
  - batch slice arrives as [128, 256] (x[p, t] = v[t*128 + p])
  - ACT: abs/ln/sin ; DVE: iterated sin powers into P [128, 256*11]
    (column t*11+j holds polar_j of batch tile t)
  - PE transposes [128, 11] slices into a [128, 128] PSUM tile at
    partition offsets 0/32/64/96 (col-group packing, 4 batch tiles)
  - K=11 matmuls read lhsT at those offsets (row-group packing) against
    the weight replicated at the same offsets; N=512 halves into PSUM
  - PSUM->SBUF copies alternate DVE/ACT; 2 MiB DMA stores
"""

import numpy as np

D = 10
EMB = 1024
N_TOTAL = 262144
N_CORES = 8
N_PER_CORE = N_TOTAL // N_CORES          # 32768
TILES_PER_CORE = N_PER_CORE // 128       # 256
SUPER = 4                                # batch tiles per super-tile (2 MiB stores)
N_SUPER = TILES_PER_CORE // SUPER        # 64
N_CHUNK = 1                              # polar-power chunks (1 = single pass)
KDIM = D + 1                             # 10 polar rows + ones row (bias)
KFAC = float(np.pi) / 85.0               # |MIN_B - MAX_B| = 85
HALF_PI = float(np.pi / 2.0)

_NC_CACHE = None
LAST_RESULTS = None


def _build_bass():
    import concourse.bacc as bacc
    import concourse.mybir as mybir
    from concourse import tile
    from concourse.masks import make_identity

    f32 = mybir.dt.float32
    f32r = mybir.dt.float32r
    AF = mybir.ActivationFunctionType
    ALU = mybir.AluOpType

    nc = bacc.Bacc("TRN2")

    xv = nc.dram_tensor("xv", [128, TILES_PER_CORE], f32, kind="ExternalInput")
    wqb = nc.dram_tensor("wqb", [128, EMB], f32, kind="ExternalInput")
    y = nc.dram_tensor("y", [N_PER_CORE, EMB], f32, kind="ExternalOutput")

    with tile.TileContext(nc) as tc:
        with (
            tc.tile_pool(name="consts", bufs=1) as consts,
            tc.tile_pool(name="work", bufs=1) as work,
            tc.tile_pool(name="lhsp", bufs=4) as lhsp,
            tc.tile_pool(name="outp", bufs=4) as outp,
            tc.tile_pool(name="ptr", bufs=2, space="PSUM") as ptr,
            tc.tile_pool(name="pout", bufs=3, space="PSUM") as pout,
        ):
            ident = consts.tile([128, 128], f32)
            make_identity(nc, ident)
            wqb_sb = consts.tile([128, EMB], f32)
            nc.sync.dma_start(wqb_sb, wqb[:])
            wqb_r = consts.tile([128, EMB], f32r)
            nc.vector.tensor_copy(wqb_r, wqb_sb)

            bias001 = consts.tile([128, 1], f32)
            nc.gpsimd.memset(bias001, 0.01)
            bias_hpi = consts.tile([128, 1], f32)
            nc.gpsimd.memset(bias_hpi, HALF_PI)

            x_sb = work.tile([128, TILES_PER_CORE], f32)
            nc.sync.dma_start(x_sb, xv[:])

            u = work.tile([128, TILES_PER_CORE], f32)
            th = work.tile([128, TILES_PER_CORE], f32)
            s = work.tile([128, TILES_PER_CORE], f32)
            c = work.tile([128, TILES_PER_CORE], f32)
            i32 = mybir.dt.int32
            nc.vector.tensor_scalar(
                u.bitcast(i32), x_sb.bitcast(i32), 0x7FFFFFFF, None,
                ALU.bitwise_and,
            )  # |x| via sign-bit clear

            def emit_trig(lo, hi):
                nc.scalar.activation(
                    th[:, lo:hi], u[:, lo:hi], AF.Ln, bias=bias001[:, :]
                )
                nc.scalar.activation(
                    s[:, lo:hi], th[:, lo:hi], AF.Sin, scale=KFAC
                )
                nc.scalar.activation(
                    c[:, lo:hi], th[:, lo:hi], AF.Sin, scale=KFAC,
                    bias=bias_hpi[:, :],
                )

            s2 = work.tile([128, TILES_PER_CORE], f32)
            s8 = work.tile([128, TILES_PER_CORE], f32)

            # P[p, t*11 + j] = polar_j(batch t*128+p); j=10 is the ones row.
            P = work.tile([128, TILES_PER_CORE * KDIM], f32)
            Pv = P.rearrange("p (t j) -> p t j", j=KDIM)

            def emit_powers(t_lo, t_hi):
                tsl = slice(t_lo, t_hi)
                sc, cc = s[:, tsl], c[:, tsl]
                s2c, s8c = s2[:, tsl], s8[:, tsl]
                Pc = Pv[:, tsl, :]
                nc.vector.tensor_mul(s2c, sc, sc)
                nc.vector.tensor_mul(s8c, s2c, s2c)     # s^4
                nc.vector.tensor_mul(s8c, s8c, s8c)     # s^8
                nc.vector.tensor_copy(Pc[:, :, 0], cc)
                for j in range(1, 9):
                    nc.vector.tensor_mul(Pc[:, :, j], Pc[:, :, j - 1], sc)
                nc.vector.tensor_mul(Pc[:, :, 9], s8c, s2c)   # s^10
                nc.vector.tensor_scalar(
                    Pc[:, :, 10], sc, 0.0, 1.0, ALU.mult, ALU.add
                )  # ones

            # Small head chunk lets PE/DMA ramp while the bulk is computed.
            HEAD_ST = 2
            emit_trig(0, HEAD_ST * SUPER)
            emit_powers(0, HEAD_ST * SUPER)
            emit_trig(HEAD_ST * SUPER, TILES_PER_CORE)

            for st in range(N_SUPER):
                if st == HEAD_ST:
                    emit_powers(HEAD_ST * SUPER, TILES_PER_CORE)
                out_sb = outp.tile([128, SUPER * EMB], f32)
                for q in range(SUPER):
                    T = st * SUPER + q
                    ptile = ptr.tile([KDIM, 128], f32)
                    nc.tensor.transpose(
                        ptile, P[:, T * KDIM : (T + 1) * KDIM], ident
                    )
                    lhs_sb = lhsp.tile([KDIM, 128], f32r)
                    if q % 2 == 0:
                        nc.vector.tensor_copy(lhs_sb, ptile)
                    else:
                        nc.scalar.copy(lhs_sb, ptile)
                    ops = pout.tile([128, EMB], f32)
                    for h in range(2):
                        nc.tensor.matmul(
                            ops[:, h * 512 : (h + 1) * 512],
                            lhsT=lhs_sb,
                            rhs=wqb_r[0:KDIM, h * 512 : (h + 1) * 512],
                            start=True,
                            stop=True,
                        )
                    dst = out_sb[:, q * EMB : (q + 1) * EMB]
                    if q % 2 == 0:
                        nc.vector.tensor_copy(dst, ops)
                    else:
                        nc.scalar.copy(dst, ops)

                rows = SUPER * 128
                yv = y[st * rows : (st + 1) * rows, :].rearrange(
                    "(q p) e -> p q e", p=128
                )
                osv = out_sb.rearrange("p (q e) -> p q e", e=EMB)
                if st >= N_SUPER - 4:
                    # Tail: smaller stores shorten the final drain chain.
                    for q in range(SUPER):
                        nc.sync.dma_start(yv[:, q : q + 1, :], osv[:, q : q + 1, :])
                else:
                    nc.sync.dma_start(yv, osv)

    nc.finalize()
    return nc


def _get_nc():
    global _NC_CACHE
    if _NC_CACHE is None:
        _NC_CACHE = _build_bass()
    return _NC_CACHE


def kernel(batch_val, Q, W, b):
    global LAST_RESULTS
    from concourse.bass_utils import run_bass_kernel_spmd

    batch_val = np.asarray(batch_val, dtype=np.float32)
    Q = np.asarray(Q, dtype=np.float32)
    W = np.asarray(W, dtype=np.float32)
    b = np.asarray(b, dtype=np.float32)

    # Fold Q and W into one [11, 1024] weight (row 10 carries the bias),
    # replicated at partition offsets 0/32/64/96 for row-group packing.
    wq = (W.astype(np.float64) @ Q.astype(np.float64)).astype(np.float32)  # [1024, 10]
    wrows = np.concatenate([wq.T, b[None, :]], axis=0)  # [11, 1024]
    wqb = np.zeros((128, EMB), dtype=np.float32)
    wqb[0:KDIM, :] = wrows

    in_maps = []
    for core in range(N_CORES):
        sl = batch_val[core * N_PER_CORE : (core + 1) * N_PER_CORE]
        xc = np.ascontiguousarray(sl.reshape(TILES_PER_CORE, 128).T)
        in_maps.append({"xv": xc, "wqb": wqb})

    nc = _get_nc()
    LAST_RESULTS = run_bass_kernel_spmd(nc, in_maps, core_ids=list(range(N_CORES)))
    return np.concatenate([r["y"] for r in LAST_RESULTS.results], axis=0)
